# revision 1
# baseline (speedup 1.0000x reference)
"""Sliding-window attention (RoPE + QKV proj + windowed softmax attention + o_proj)
for Trainium2, SPMD over 8 NeuronCores.

Sharding: batch (2) x head-groups (4 groups of 4 heads) -> 8 cores.
Each core computes qkv for its 4 heads, windowed attention, and a partial
o_proj (its heads' columns of w_o); host sums the 4 partials per batch.

Matmuls run in fp32r (rounded fp32, ~2e-4 relerr, 4x the fp32 rate) with fp32
PSUM accumulation.
"""
import sys

sys.path.insert(0, "/opt/trn_rl_repo")

import numpy as np

B = 2
S = 2048
HIDDEN = 2048
N_HEADS = 16
DH = 128
WINDOW = 512
HPC = 4  # heads per core
N_CORES = 8
QKV_O = 3 * HPC * DH  # 1536
SCALE = 1.0 / np.sqrt(DH)
NEG = -1.0e30

_CACHE = {}

# tunable knobs (sim A/B)
CFG = {
    "psq_bufs": 6, "psv_bufs": 2,
    "xt_extra": 2, "rope_bufs": 3, "evac_bufs": 3, "tbl_bufs": 2,
    "pm_bufs": 6, "pr_bufs": 6, "pt_bufs": 4, "qk_bufs": 3,
    "pss_bufs": 2, "pst_bufs": 1, "pso_bufs": 2, "wo_prefetch": False,
    "psc_bufs": 4, "ob_bufs": 6,
    "phases": "ABC",
    "pv": "strip",
    "strip_bufs": 9, "merge_bc": True,
    "spill_eng": "sync", "out_eng": "sync", "hpair": False,
    "batch_strip": True, "strip_bufs2": 1, "c_alt_evac": False,
}


def _build_module(repeat=1, cfg=None):
    cfg = {**CFG, **(cfg or {})}
    import concourse.tile as tile
    from concourse import bacc, mybir
    from contextlib import ExitStack

    f32 = mybir.dt.float32
    f32r = mybir.dt.float32r
    bf16 = mybir.dt.bfloat16
    AF = mybir.ActivationFunctionType

    nc = bacc.Bacc("TRN2", target_bir_lowering=False, debug=False)

    xT = nc.declare_dram_parameter("xT", [HIDDEN, S], f32, isOutput=False)
    wT = nc.declare_dram_parameter("wT", [HIDDEN, QKV_O], f32, isOutput=False)
    woT = nc.declare_dram_parameter("woT", [HPC * DH, HIDDEN], f32, isOutput=False)
    cosq = nc.declare_dram_parameter("cosq", [DH, S], f32, isOutput=False)
    sinq = nc.declare_dram_parameter("sinq", [DH, S], f32, isOutput=False)
    cosk = nc.declare_dram_parameter("cosk", [DH, S], f32, isOutput=False)
    sink = nc.declare_dram_parameter("sink", [DH, S], f32, isOutput=False)
    mask_d = nc.declare_dram_parameter("mask", [128, 640], bf16, isOutput=False)
    idnb_d = nc.declare_dram_parameter("idnb", [128, 128], bf16, isOutput=False)
    zeros_d = nc.declare_dram_parameter("zeros", [128, 512], f32, isOutput=False)
    idn_d = nc.declare_dram_parameter("idn", [128, 128], f32, isOutput=False)
    out_d = nc.declare_dram_parameter("out", [S, HIDDEN], f32, isOutput=True)

    NKT = HIDDEN // 128  # 16 contraction chunks
    NSC = S // 512  # 4 sequence chunks
    NST = S // 128  # 16 sequence tiles

    with tile.TileContext(nc) as tc, ExitStack() as top:
        dram = top.enter_context(tc.tile_pool(name="dram", bufs=1, space="DRAM"))
        qk_rot = dram.tile([2 * HPC * DH, S], f32r, tag="qkrot")
        v_sd = dram.tile([S, HPC * DH], f32r, tag="vsd")

        cpool = top.enter_context(tc.tile_pool(name="consts", bufs=1))
        msk = cpool.tile([128, 640], bf16, tag="mask")
        nc.sync.dma_start(msk[:], mask_d[:])
        idnb = cpool.tile([128, 128], bf16, tag="idnb")
        nc.sync.dma_start(idnb[:], idnb_d[:])
        idn = cpool.tile([128, 128], f32r, tag="idn")
        nc.sync.dma_start(idn[:], idn_d[:].bitcast(f32r))


        attn_pool = top.enter_context(tc.tile_pool(name="attn", bufs=HPC))
        wo_tiles = []
        if cfg["wo_prefetch"] and "C" in cfg["phases"]:
            wo_pool = top.enter_context(tc.tile_pool(name="wo", bufs=HPC))
            for h in range(HPC):
                t = wo_pool.tile([128, HIDDEN], f32r, tag="wo")
                nc.sync.dma_start(
                    t[:], woT[h * 128 : (h + 1) * 128, :].bitcast(f32r)
                )
                wo_tiles.append(t)

        for rep in range(repeat):
            # ------------- Phase A: QKV projection + RoPE -------------
            if "A" in cfg["phases"]:
              with ExitStack() as ph:
                wt_pool = ph.enter_context(tc.tile_pool(name="wt", bufs=NKT))
                xt_pool = ph.enter_context(tc.tile_pool(name="xt", bufs=NKT + cfg["xt_extra"]))
                tbl_pool = ph.enter_context(tc.tile_pool(name="tbl", bufs=cfg["tbl_bufs"]))
                rope_pool = ph.enter_context(tc.tile_pool(name="rope", bufs=cfg["rope_bufs"]))
                evac_pool = ph.enter_context(tc.tile_pool(name="evac", bufs=cfg["evac_bufs"]))
                psq_pool = ph.enter_context(
                    tc.tile_pool(name="psq", bufs=cfg["psq_bufs"], space="PSUM")
                )
                psv_pool = ph.enter_context(
                    tc.tile_pool(name="psv", bufs=cfg["psv_bufs"], space="PSUM")
                )

                wt_tiles = []
                for k in range(NKT):
                    t = wt_pool.tile([128, QKV_O], f32r, tag="wt")
                    nc.sync.dma_start(
                        t[:], wT[k * 128 : (k + 1) * 128, :].bitcast(f32r)
                    )
                    wt_tiles.append(t)

                for sc in range(NSC):
                    s0 = sc * 512
                    xt_tiles = []
                    for k in range(NKT):
                        t = xt_pool.tile([128, 512], f32r, tag="xt")
                        nc.sync.dma_start(
                            t[:],
                            xT[k * 128 : (k + 1) * 128, s0 : s0 + 512].bitcast(f32r),
                        )
                        xt_tiles.append(t)

                    tb = {}
                    for nm, src in (
                        ("cosq", cosq),
                        ("sinq", sinq),
                        ("cosk", cosk),
                        ("sink", sink),
                    ):
                        t = tbl_pool.tile([128, 512], f32, tag=nm)
                        nc.sync.dma_start(t[:], src[:, s0 : s0 + 512])
                        tb[nm] = t

                    # QKV accumulation in k-interleaved waves so PE can
                    # consume wt/xt tiles as the DMA delivers them (cuts the
                    # startup stall on chunk 0). Wave = up to 8 live psum
                    # accumulators (q/k groups + v s-tiles).
                    def rope_evac(t_o, ps):
                        ct = tb["cosq"] if t_o < HPC else tb["cosk"]
                        st = tb["sinq"] if t_o < HPC else tb["sink"]
                        tmp = rope_pool.tile([128, 512], f32, tag="tmp")
                        nc.vector.tensor_mul(tmp[0:64, :], ps[64:128, :], st[0:64, :])
                        nc.vector.tensor_mul(
                            tmp[64:128, :], ps[0:64, :], st[64:128, :]
                        )
                        qc = rope_pool.tile([128, 512], f32, tag="qc")
                        nc.vector.tensor_mul(qc[:], ps[:], ct[:])
                        ro = evac_pool.tile([128, 512], f32r, tag="ro")
                        nc.vector.tensor_add(ro[:], qc[:], tmp[:])
                        getattr(nc, cfg["spill_eng"]).dma_start(
                            qk_rot[t_o * 128 : (t_o + 1) * 128, s0 : s0 + 512],
                            ro[:],
                        )

                    def v_evac(st_i, psv):
                        vo = evac_pool.tile([128, 512], f32r, tag="vo")
                        nc.vector.tensor_copy(vo[:], psv[:])
                        r0 = (sc * 4 + st_i) * 128
                        getattr(nc, cfg["spill_eng"]).dma_start(
                            v_sd[r0 : r0 + 128, :], vo[:]
                        )

                    for t_o in range(2 * HPC):
                        ps = psq_pool.tile([128, 512], f32, tag="psq")
                        for k in range(NKT):
                            nc.tensor.matmul(
                                ps[:],
                                wt_tiles[k][:, t_o * 128 : (t_o + 1) * 128],
                                xt_tiles[k][:],
                                start=(k == 0),
                                stop=(k == NKT - 1),
                            )
                        rope_evac(t_o, ps)
                    for st_i in range(4):
                        psv = psv_pool.tile([128, 512], f32, tag="psv")
                        for k in range(NKT):
                            nc.tensor.matmul(
                                psv[:],
                                xt_tiles[k][:, st_i * 128 : (st_i + 1) * 128],
                                wt_tiles[k][:, 2 * HPC * 128 : 3 * HPC * 128],
                                start=(k == 0),
                                stop=(k == NKT - 1),
                            )
                        v_evac(st_i, psv)

            # ------------- Phase C body (emitted merged or standalone) ----
            attn_tiles = []

            def emit_phase_c(ph):
                ob_pool = ph.enter_context(
                    tc.tile_pool(name="ob", bufs=cfg["ob_bufs"])
                )
                psc_pool = ph.enter_context(
                    tc.tile_pool(name="psc", bufs=cfg["psc_bufs"], space="PSUM")
                )
                if cfg["wo_prefetch"]:
                    wts = wo_tiles
                else:
                    wo_pool = ph.enter_context(tc.tile_pool(name="wo", bufs=HPC))
                    wts = []
                    for h in range(HPC):
                        t = wo_pool.tile([128, HIDDEN], f32r, tag="wo")
                        nc.sync.dma_start(
                            t[:], woT[h * 128 : (h + 1) * 128, :].bitcast(f32r)
                        )
                        wts.append(t)
                for st_i in range(NST):
                    for mc in range(HIDDEN // 512):
                        ps = psc_pool.tile([128, 512], f32, tag="psc")
                        for h in range(HPC):
                            nc.tensor.matmul(
                                ps[:],
                                attn_tiles[h][:, st_i * 128 : (st_i + 1) * 128],
                                wts[h][:, mc * 512 : (mc + 1) * 512],
                                start=(h == 0),
                                stop=(h == HPC - 1),
                            )
                        ob = ob_pool.tile([128, 512], f32, tag="ob")
                        if cfg["c_alt_evac"] and (st_i * 4 + mc) % 2 == 1:
                            nc.scalar.copy(ob[:], ps[:])
                        else:
                            nc.vector.tensor_copy(ob[:], ps[:])
                        getattr(nc, cfg["out_eng"]).dma_start(
                            out_d[
                                st_i * 128 : (st_i + 1) * 128,
                                mc * 512 : (mc + 1) * 512,
                            ],
                            ob[:],
                        )

            # ------------- Phase B: windowed attention -------------
            if "B" in cfg["phases"]:
              with ExitStack() as ph:
                qk_pool = ph.enter_context(tc.tile_pool(name="qk", bufs=cfg["qk_bufs"]))
                v_pool = ph.enter_context(
                    tc.tile_pool(name="vt", bufs=(2 * NST + 2) if cfg["hpair"] else NST + 2)
                )
                pm_pool = ph.enter_context(tc.tile_pool(name="pm", bufs=cfg["pm_bufs"]))
                pr_pool = ph.enter_context(tc.tile_pool(name="pr", bufs=cfg["pr_bufs"]))
                pt_pool = ph.enter_context(tc.tile_pool(name="pt", bufs=cfg["pt_bufs"]))
                sm_pool = ph.enter_context(tc.tile_pool(name="sm", bufs=cfg.get("sm_bufs", 6)))
                phps = ExitStack()
                pss_pool = phps.enter_context(
                    tc.tile_pool(name="pss", bufs=cfg["pss_bufs"], space="PSUM")
                )
                pst_pool = phps.enter_context(
                    tc.tile_pool(name="pst", bufs=cfg["pst_bufs"], space="PSUM")
                )
                pso_pool = phps.enter_context(
                    tc.tile_pool(name="pso", bufs=cfg["pso_bufs"], space="PSUM")
                )

                strip_pool = (
                    ph.enter_context(tc.tile_pool(name="strip", bufs=1 if cfg["batch_strip"] else cfg["strip_bufs"]))
                    if cfg["pv"] == "strip"
                    else None
                )
                zeros = None
                if cfg["pv"] == "strip":
                    zp_pool = ph.enter_context(tc.tile_pool(name="zp", bufs=1))
                    zeros = zp_pool.tile([128, 512], f32r, tag="zeros")
                    nc.sync.dma_start(zeros[:], zeros_d[:].bitcast(f32r))
                # last q-block index whose PV contribution lands in psum bank bk
                FIRST_BANK = {0: 0, 1: 0, 2: 4, 3: 8}
                LAST_BANK = {0: 3, 1: 7, 2: 11, 3: 15}

                def setup_head(h):
                    qh = qk_pool.tile([128, S], f32r, tag="qh", name=f"qh{h}")
                    kh = qk_pool.tile([128, S], f32r, tag="kh", name=f"kh{h}")
                    for sc in range(NSC):
                        s0 = sc * 512
                        nc.sync.dma_start(
                            qh[:, s0 : s0 + 512],
                            qk_rot[h * 128 : (h + 1) * 128, s0 : s0 + 512],
                        )
                        nc.sync.dma_start(
                            kh[:, s0 : s0 + 512],
                            qk_rot[
                                (HPC + h) * 128 : (HPC + h + 1) * 128, s0 : s0 + 512
                            ],
                        )
                    vt = []
                    for jb in range(NST):
                        t = v_pool.tile(
                            [128, 128], f32r, tag="vt", name=f"vt{h}_{jb}"
                        )
                        nc.sync.dma_start(
                            t[:],
                            v_sd[jb * 128 : (jb + 1) * 128, h * 128 : (h + 1) * 128],
                        )
                        vt.append(t)
                    ah = attn_pool.tile([128, S], f32r, tag="ah", name=f"ah{h}")
                    attn_tiles.append(ah)
                    # piece-granular PV bookkeeping
                    pieces_by_ready = {}
                    for jb in range(NST):
                        w0, w1 = jb * 128, min(jb * 128 + 640, S)
                        c = w0
                        while c < w1:
                            nxt = min(w1, (c // 512 + 1) * 512)
                            pieces_by_ready.setdefault(
                                min(jb + 4, NST - 1), []
                            ).append((jb, c, nxt))
                            c = nxt
                    roll = None
                    if cfg["batch_strip"]:
                        roll = strip_pool.tile(
                            [128, 8 * 640], f32r, tag="roll", name=f"roll{h}"
                        )
                    return dict(
                        h=h, qh=qh, kh=kh, vt=vt, ah=ah,
                        pv_banks=[None] * 4, strips=[None] * NST,
                        pieces=pieces_by_ready, roll=roll,
                    )

                def strip_ap(st, jb, c0, c1):
                    # columns [c0, c1) of key-block jb's strip
                    if cfg["batch_strip"]:
                        base = (jb % 8) * 640
                        return st["roll"][:, base + c0 : base + c1]
                    return st["strips"][jb][:, c0:c1]

                def emit_pv_pieces(st, i):
                    h, vt, ah = st["h"], st["vt"], st["ah"]
                    pv_banks = st["pv_banks"]
                    for jb, c, nxt in st["pieces"].get(i, ()):
                        bk = c // 512
                        if pv_banks[bk] is None:
                            pv_banks[bk] = pso_pool.tile(
                                [128, 512], f32, tag="pvo",
                                name=f"pvo_h{h}_b{bk}",
                            )
                            # zero the bank so all PV pieces accumulate
                            nc.tensor.matmul(
                                pv_banks[bk][:],
                                idn[:],
                                zeros[:],
                                start=True,
                                stop=False,
                                skip_group_check=True,
                            )
                        last = LAST_BANK[bk] == jb
                        nc.tensor.matmul(
                            pv_banks[bk][:, c - bk * 512 : nxt - bk * 512],
                            vt[jb][:],
                            strip_ap(st, jb, c - jb * 128, nxt - jb * 128),
                            start=False,
                            stop=last,
                            skip_group_check=True,
                        )
                        if last:
                            nc.vector.tensor_copy(
                                ah[:, bk * 512 : (bk + 1) * 512],
                                pv_banks[bk][:],
                            )

                def process_block(st, i):
                    h, qh, kh = st["h"], st["qh"], st["kh"]
                    strips = st["strips"]
                    jlo = max(0, i * 128 - WINDOW)
                    w = i * 128 + 128 - jlo
                    nblk = w // 128
                    # place the window at offset 128 in a 2-bank psum tile so
                    # the 640-wide case splits 384+256 (both >=256 -> 1 cyc/row
                    # for fp32r) instead of 512+128. w==512 uses offset 0 (one
                    # bank-aligned piece).
                    off = 0 if w == 512 else 128
                    ps_s = pss_pool.tile([128, 1024], f32, tag="pss")
                    mo = 640 - w
                    c = off
                    while c < off + w:
                        nxt = min(off + w, (c // 512 + 1) * 512)
                        # mask preloaded into psum by PE, scores accumulate
                        nc.tensor.matmul(
                            ps_s[:, c:nxt],
                            idnb[:],
                            msk[:, mo + c - off : mo + nxt - off],
                            start=True,
                            stop=False,
                            skip_group_check=True,
                        )
                        nc.tensor.matmul(
                            ps_s[:, c:nxt],
                            qh[:, i * 128 : (i + 1) * 128],
                            kh[:, jlo + c - off : jlo + nxt - off],
                            start=False,
                            stop=True,
                            skip_group_check=True,
                        )
                        c = nxt
                    pm = pm_pool.tile([128, 640], f32, tag="pm")
                    sums = sm_pool.tile([128, 1], f32, tag="sums")
                    nc.scalar.activation(
                        pm[:, :w], ps_s[:, off : off + w], AF.Exp, accum_out=sums[:]
                    )
                    rc = sm_pool.tile([128, 1], f32, tag="rc")
                    nc.vector.reciprocal(rc[:], sums[:])
                    pr = pr_pool.tile([128, 640], f32r, tag="pr")
                    nc.vector.tensor_scalar_mul(pr[:, :w], pm[:, :w], rc[:])
                    j0 = jlo // 128
                    if cfg["batch_strip"]:
                        # all nblk transposes into one psum tile, then one (or
                        # two, on slot-wrap) strided DVE copies into the
                        # rolling strip buffer. start=True only on each psum
                        # bank's first slice so earlier slices aren't
                        # re-zeroed.
                        ps_t = pst_pool.tile([128, 640], f32r, tag="pst")
                        for z in range(nblk):
                            nc.tensor.matmul(
                                ps_t[:, z * 128 : (z + 1) * 128],
                                pr[:, z * 128 : (z + 1) * 128],
                                idn[:],
                                is_transpose=True,
                                start=(z == 0 or z == 4),
                                stop=(z == min(nblk, 4) - 1 or z == nblk - 1),
                                skip_group_check=True,
                            )
                        roll = st["roll"]
                        # dest col for z: ((j0+z)%8)*640 + (i-j0-z)*128
                        # = base + z*512 within a non-wrapping slot segment,
                        # which is axis-aligned in a (a=col/512, b=(col%512)/128)
                        # view of the rolling buffer.
                        roll4 = roll[:].rearrange("p (a b c) -> p a b c", b=4, c=128)
                        ps4 = ps_t[:].rearrange("p (z o c) -> p z o c", o=1, c=128)
                        z = 0
                        while z < nblk:
                            s0 = (j0 + z) % 8
                            zlen = min(nblk - z, 8 - s0)
                            base = s0 * 640 + (i - j0 - z) * 128
                            a0, b0 = base // 512, (base % 512) // 128
                            nc.vector.tensor_copy(
                                roll4[:, a0 : a0 + zlen, b0 : b0 + 1, :],
                                ps4[:, z : z + zlen, :, :],
                            )
                            z += zlen
                    else:
                        for z in range(nblk):
                            jb = j0 + z
                            if strips[jb] is None:
                                strips[jb] = strip_pool.tile(
                                    [128, 640], f32r, tag="strip",
                                    name=f"strip_h{h}_j{jb}",
                                )
                            ps_t = pst_pool.tile([128, 128], f32r, tag="pst")
                            nc.tensor.transpose(
                                ps_t[:], pr[:, z * 128 : (z + 1) * 128], idn[:]
                            )
                            nc.vector.tensor_copy(
                                strips[jb][:, (i - jb) * 128 : (i - jb + 1) * 128],
                                ps_t[:],
                            )
                    emit_pv_pieces(st, i)

                if cfg["hpair"]:
                    for hp in range(HPC // 2):
                        states = [setup_head(2 * hp), setup_head(2 * hp + 1)]
                        for i in range(NST):
                            for st in states:
                                process_block(st, i)
                else:
                    for h in range(HPC):
                        st = setup_head(h)
                        for i in range(NST):
                            process_block(st, i)




                phps.close()
                if cfg["merge_bc"] and "C" in cfg["phases"]:
                    emit_phase_c(ph)

            # ------------- Phase C: output projection (standalone) -------------
            if "C" in cfg["phases"] and not cfg["merge_bc"]:
                with ExitStack() as ph:
                    emit_phase_c(ph)

    nc.compile()
    return nc


def _get_module(repeat=1, cfg=None):
    key = ("nc", repeat, tuple(sorted((cfg or {}).items())))
    if key not in _CACHE:
        _CACHE[key] = _build_module(repeat, cfg)
    return _CACHE[key]


def make_in_maps(hidden_states, cos, sin, w_qkv, w_o):
    hidden_states = np.asarray(hidden_states, dtype=np.float32)
    cos = np.asarray(cos, dtype=np.float32)
    sin = np.asarray(sin, dtype=np.float32)
    w_qkv = np.asarray(w_qkv, dtype=np.float32)
    w_o = np.asarray(w_o, dtype=np.float32)

    cosT = np.ascontiguousarray(cos.T)  # [DH, S]
    sinT = np.ascontiguousarray(sin.T)
    sinS = sinT.copy()
    sinS[: DH // 2] *= -1.0  # fold rotate_half sign
    cq = (cosT * SCALE).astype(np.float32)
    sq = (sinS * SCALE).astype(np.float32)
    ck = cosT.astype(np.float32)
    sk = sinS.astype(np.float32)

    import ml_dtypes

    qi = np.arange(128)[:, None]
    jj = np.arange(640)[None, :]
    mask = np.where((jj > qi) & (jj <= qi + WINDOW), 0.0, NEG).astype(
        ml_dtypes.bfloat16
    )
    idn = np.eye(128, dtype=np.float32)
    idnb = np.eye(128, dtype=ml_dtypes.bfloat16)

    xTs = [np.ascontiguousarray(hidden_states[b].T) for b in range(B)]

    in_maps = []
    for c in range(N_CORES):
        b, hg = divmod(c, N_CORES // B)
        r0 = hg * HPC * DH
        wq = w_qkv[r0 : r0 + HPC * DH]
        wk = w_qkv[N_HEADS * DH + r0 : N_HEADS * DH + r0 + HPC * DH]
        wv = w_qkv[2 * N_HEADS * DH + r0 : 2 * N_HEADS * DH + r0 + HPC * DH]
        wTc = np.ascontiguousarray(np.concatenate([wq, wk, wv], axis=0).T)
        woTc = np.ascontiguousarray(w_o[:, r0 : r0 + HPC * DH].T)
        in_maps.append(
            {
                "xT": xTs[b],
                "wT": wTc,
                "woT": woTc,
                "cosq": cq,
                "sinq": sq,
                "cosk": ck,
                "sink": sk,
                "mask": mask,
                "idn": idn,
                "idnb": idnb,
                "zeros": np.zeros((128, 512), dtype=np.float32),
            }
        )
    return in_maps


def gather(results):
    out = np.zeros((B, S, HIDDEN), dtype=np.float32)
    for c in range(N_CORES):
        b = c // (N_CORES // B)
        out[b] += results[c]["out"]
    return out


def kernel(hidden_states, cos, sin, w_qkv, w_o):
    from concourse.bass_utils import run_bass_kernel_spmd

    nc = _get_module()
    in_maps = make_in_maps(hidden_states, cos, sin, w_qkv, w_o)
    res = run_bass_kernel_spmd(nc, in_maps, list(range(N_CORES)))
    return gather(res.results)



# revision 9
# speedup vs baseline: 8.0198x; 8.0198x over previous
"""Sliding-window attention (RoPE + QKV proj + windowed softmax attention + o_proj)
for Trainium2, SPMD over 8 NeuronCores.

Sharding: batch (2) x head-groups (4 groups of 4 heads) -> 8 cores.
Each core computes qkv for its 4 heads, windowed attention, and a partial
o_proj (its heads' columns of w_o); host sums the 4 partials per batch.

v2 design (vs f32r baseline):
- fp16 everywhere on the PE (1 cyc/row at any free size; products exact in
  f32 PSUM). Host converts x/w to fp16; rel err ~5e-4 vs 2e-2 tolerance.
- No DRAM spill: rotated q/k ([dh, S] layout) and v ([S, dh]) live in SBUF
  between phases (6 MB in fp16).
- Attention computes S^T = K^T-slices x Q-slices directly per 128x128 cell:
  no PE transposes, no full-width mask preloads (only 2 boundary cells per
  q-block get a mask matmul), no PSUM zeroing. exp() -> fp16 probs; softmax
  denominators via an all-ones matmul riding the same prob tiles; normalize
  on DVE at PV-evac time.
"""
import sys

sys.path.insert(0, "/opt/trn_rl_repo")

import numpy as np

B = 2
S = 2048
HIDDEN = 2048
N_HEADS = 16
DH = 128
WINDOW = 512
HPC = 4  # heads per core
N_CORES = 8
QKV_O = 3 * HPC * DH  # 1536
SCALE = 1.0 / np.sqrt(DH)
NEG = -30000.0  # fp16-safe -inf; exp() underflows to exactly 0

_CACHE = {}

# tunable knobs (sim A/B)
CFG = {
    "psq_bufs": 6, "psv_bufs": 2,
    "xt_extra": 12, "rope_bufs": 3, "tbl_bufs": 2,
    "score_bufs": 3, "pvs_bufs": 2, "p_bufs": 3, "rc_bufs": 3,
    "pipe": 2,
    "psc_bufs": 4, "ob_bufs": 6,
    "phases": "ABC",
    "dump": False,
}


def _build_module(repeat=1, cfg=None):
    cfg = {**CFG, **(cfg or {})}
    import concourse.tile as tile
    from concourse import bacc, mybir
    from contextlib import ExitStack

    f32 = mybir.dt.float32
    f16 = mybir.dt.float16
    AF = mybir.ActivationFunctionType

    nc = bacc.Bacc("TRN2", target_bir_lowering=False, debug=False)

    xT = nc.declare_dram_parameter("xT", [HIDDEN, S], f16, isOutput=False)
    wT = nc.declare_dram_parameter("wT", [HIDDEN, QKV_O], f16, isOutput=False)
    woT = nc.declare_dram_parameter("woT", [HPC * DH, HIDDEN], f16, isOutput=False)
    cosq = nc.declare_dram_parameter("cosq", [DH, S], f32, isOutput=False)
    sinq = nc.declare_dram_parameter("sinq", [DH, S], f32, isOutput=False)
    cosk = nc.declare_dram_parameter("cosk", [DH, S], f32, isOutput=False)
    sink = nc.declare_dram_parameter("sink", [DH, S], f32, isOutput=False)
    mdiag_d = nc.declare_dram_parameter("mdiag", [128, 128], f16, isOutput=False)
    mwend_d = nc.declare_dram_parameter("mwend", [128, 128], f16, isOutput=False)
    idn_d = nc.declare_dram_parameter("idn", [128, 128], f16, isOutput=False)
    ones_d = nc.declare_dram_parameter("ones", [128, 128], f16, isOutput=False)
    out_d = nc.declare_dram_parameter("out", [S, HIDDEN], f16, isOutput=True)
    if cfg["dump"]:
        dbg_qh = nc.declare_dram_parameter("dbg_qh", [HPC * 128, S], f16, isOutput=True)
        dbg_kh = nc.declare_dram_parameter("dbg_kh", [HPC * 128, S], f16, isOutput=True)
        dbg_v = nc.declare_dram_parameter("dbg_v", [S, HPC * DH], f16, isOutput=True)
        dbg_ah = nc.declare_dram_parameter("dbg_ah", [HPC * 128, S], f16, isOutput=True)

    NKT = HIDDEN // 128  # 16 contraction chunks
    NSC = S // 512  # 4 sequence chunks
    NST = S // 128  # 16 sequence tiles

    with tile.TileContext(nc) as tc, ExitStack() as top:
        cpool = top.enter_context(tc.tile_pool(name="consts", bufs=1))
        mdiag = cpool.tile([128, 128], f16, tag="mdiag")
        nc.sync.dma_start(mdiag[:], mdiag_d[:])
        mwend = cpool.tile([128, 128], f16, tag="mwend")
        nc.sync.dma_start(mwend[:], mwend_d[:])
        idn = cpool.tile([128, 128], f16, tag="idn")
        nc.sync.dma_start(idn[:], idn_d[:])
        ones = cpool.tile([128, 128], f16, tag="ones")
        nc.sync.dma_start(ones[:], ones_d[:])

        # persistent qkv/attn tiles (SBUF-resident between phases)
        perpool = top.enter_context(tc.tile_pool(name="qkv", bufs=1))

        for rep in range(repeat):
            qh = [perpool.tile([128, S], f16, tag=f"qh{h}", name=f"qh{h}")
                  for h in range(HPC)]
            kh = [perpool.tile([128, S], f16, tag=f"kh{h}", name=f"kh{h}")
                  for h in range(HPC)]
            v16 = [perpool.tile([128, HPC * DH], f16, tag=f"v{j}", name=f"v{j}")
                   for j in range(NST)]
            ah = [perpool.tile([128, S], f16, tag=f"ah{h}", name=f"ah{h}")
                  for h in range(HPC)]

            # ------------- Phase A: QKV projection + RoPE -------------
            if "A" in cfg["phases"]:
              with ExitStack() as ph:
                wt_pool = ph.enter_context(tc.tile_pool(name="wt", bufs=NKT))
                xt_pool = ph.enter_context(
                    tc.tile_pool(name="xt", bufs=NKT + cfg["xt_extra"])
                )
                tbl_pool = ph.enter_context(
                    tc.tile_pool(name="tbl", bufs=cfg["tbl_bufs"])
                )
                rope_pool = ph.enter_context(
                    tc.tile_pool(name="rope", bufs=cfg["rope_bufs"])
                )
                # single psum ring shared by q/k chains and v chains
                ps_pool = ph.enter_context(
                    tc.tile_pool(name="psa", bufs=8, space="PSUM")
                )

                # interleave wt / first-chunk xt DMAs so sc=0's k-interleaved
                # matmul waves can start as soon as chunk 0 lands
                wt_tiles = []
                xt0_tiles = []
                for k in range(NKT):
                    t = wt_pool.tile([128, QKV_O], f16, tag="wt")
                    nc.sync.dma_start(t[:], wT[k * 128 : (k + 1) * 128, :])
                    wt_tiles.append(t)
                    x = xt_pool.tile([128, 512], f16, tag="xt")
                    nc.sync.dma_start(x[:], xT[k * 128 : (k + 1) * 128, 0:512])
                    xt0_tiles.append(x)

                for sc in range(NSC):
                    s0 = sc * 512
                    if sc == 0:
                        xt_tiles = xt0_tiles
                    else:
                        xt_tiles = []
                        for k in range(NKT):
                            t = xt_pool.tile([128, 512], f16, tag="xt")
                            nc.sync.dma_start(
                                t[:], xT[k * 128 : (k + 1) * 128, s0 : s0 + 512]
                            )
                            xt_tiles.append(t)

                    tb = {}
                    for nm, src in (
                        ("cosq", cosq),
                        ("sinq", sinq),
                        ("cosk", cosk),
                        ("sink", sink),
                    ):
                        t = tbl_pool.tile([128, 512], f32, tag=nm)
                        nc.sync.dma_start(t[:], src[:, s0 : s0 + 512])
                        tb[nm] = t

                    def rope_evac(t_o, ps):
                        ct = tb["cosq"] if t_o < HPC else tb["cosk"]
                        st = tb["sinq"] if t_o < HPC else tb["sink"]
                        tmp = rope_pool.tile([128, 512], f32, tag="tmp")
                        nc.vector.tensor_mul(tmp[0:64, :], ps[64:128, :], st[0:64, :])
                        nc.vector.tensor_mul(
                            tmp[64:128, :], ps[0:64, :], st[64:128, :]
                        )
                        qc = rope_pool.tile([128, 512], f32, tag="qc")
                        nc.vector.tensor_mul(qc[:], ps[:], ct[:])
                        dst = qh[t_o] if t_o < HPC else kh[t_o - HPC]
                        nc.vector.tensor_add(dst[:, s0 : s0 + 512], qc[:], tmp[:])

                    qk_ps = [
                        ps_pool.tile([128, 512], f32, tag="ps", name=f"psqk{sc}_{t}")
                        for t in range(2 * HPC)
                    ]
                    if sc == 0:
                        # k-interleaved: consume each (wt, xt) chunk across all
                        # 8 chains as the DMA delivers it
                        for k in range(NKT):
                            last = k == NKT - 1
                            for t_o in range(2 * HPC):
                                nc.tensor.matmul(
                                    qk_ps[t_o][:],
                                    wt_tiles[k][:, t_o * 128 : (t_o + 1) * 128],
                                    xt_tiles[k][:],
                                    start=(k == 0),
                                    stop=last,
                                )
                                if last:
                                    rope_evac(t_o, qk_ps[t_o])
                    else:
                        # steady state: chain-sequential, evacs stagger
                        for t_o in range(2 * HPC):
                            for k in range(NKT):
                                nc.tensor.matmul(
                                    qk_ps[t_o][:],
                                    wt_tiles[k][:, t_o * 128 : (t_o + 1) * 128],
                                    xt_tiles[k][:],
                                    start=(k == 0),
                                    stop=(k == NKT - 1),
                                )
                            rope_evac(t_o, qk_ps[t_o])
                    for st_i in range(4):
                        psv = ps_pool.tile(
                            [128, 512], f32, tag="ps", name=f"psv{sc}_{st_i}"
                        )
                        for k in range(NKT):
                            nc.tensor.matmul(
                                psv[:],
                                xt_tiles[k][:, st_i * 128 : (st_i + 1) * 128],
                                wt_tiles[k][:, 2 * HPC * 128 : 3 * HPC * 128],
                                start=(k == 0),
                                stop=(k == NKT - 1),
                            )
                        nc.vector.tensor_copy(v16[sc * 4 + st_i][:], psv[:])

            # ------------- Phase B: windowed attention (S^T cells) -------
            if "B" in cfg["phases"]:
              with ExitStack() as ph:
                wo_pool = ph.enter_context(tc.tile_pool(name="wo", bufs=HPC))
                wo_tiles = []
                for h in range(HPC):
                    t = wo_pool.tile([128, HIDDEN], f16, tag="wo")
                    nc.sync.dma_start(t[:], woT[h * 128 : (h + 1) * 128, :])
                    wo_tiles.append(t)

                p_pool = ph.enter_context(
                    tc.tile_pool(name="pp", bufs=cfg["p_bufs"])
                )
                rc_pool = ph.enter_context(
                    tc.tile_pool(name="rc", bufs=cfg["rc_bufs"])
                )
                phps = ExitStack()
                score_pool = phps.enter_context(
                    tc.tile_pool(name="score", bufs=cfg["score_bufs"], space="PSUM")
                )
                pvs_pool = phps.enter_context(
                    tc.tile_pool(name="pvs", bufs=cfg["pvs_bufs"], space="PSUM")
                )

                def emit_qk(h, i):
                    """Score cells S^T[j, q] for q-block i, all window j-blocks.

                    PSUM start=True pending-zeroes the whole 2KB bank, so each
                    bank gets exactly ONE start (its first matmul); later cells
                    in the bank are zeroed lazily on first touch.
                    """
                    j0 = max(0, i - 4)
                    nblk = i - j0 + 1
                    ps = score_pool.tile([128, 640], f32, tag="score")
                    qsl = qh[h][:, i * 128 : (i + 1) * 128]
                    started = [False, False]  # bank 0: cells 0-3, bank 1: cell 4
                    for z in range(nblk):
                        jb = j0 + z
                        cell = ps[:, z * 128 : (z + 1) * 128]
                        bk = z // 4
                        last_in_bank = z == nblk - 1 or (z == 3 and nblk > 4)
                        msk = None
                        if jb == i:
                            msk = mdiag
                        elif jb == i - 4:
                            msk = mwend
                        if msk is not None:
                            nc.tensor.matmul(
                                cell, idn[:], msk[:],
                                start=not started[bk], stop=False,
                                skip_group_check=True,
                            )
                            started[bk] = True
                        nc.tensor.matmul(
                            cell,
                            kh[h][:, jb * 128 : (jb + 1) * 128],
                            qsl,
                            start=not started[bk],
                            stop=last_in_bank,
                            skip_group_check=True,
                        )
                        started[bk] = True
                    return ps, nblk, j0

                def emit_exp(ps, nblk):
                    w = nblk * 128
                    pt = p_pool.tile([128, 640], f16, tag="p")
                    nc.scalar.activation(pt[:, :w], ps[:, :w], AF.Exp)
                    return pt

                def emit_pv(h, i, pt, nblk, j0):
                    # PV accum (cols 0:128) and softmax sums (cols 128:256)
                    # share one bank: single start on the first matmul only.
                    pvs = pvs_pool.tile([128, 256], f32, tag="pvs")
                    for z in range(nblk):
                        jb = j0 + z
                        psl = pt[:, z * 128 : (z + 1) * 128]
                        nc.tensor.matmul(
                            pvs[:, 0:128],
                            v16[jb][:, h * 128 : (h + 1) * 128],
                            psl,
                            start=(z == 0), stop=False,
                            skip_group_check=True,
                        )
                        nc.tensor.matmul(
                            pvs[:, 128:256],
                            ones[:],
                            psl,
                            start=False, stop=(z == nblk - 1),
                            skip_group_check=True,
                        )
                    rc = rc_pool.tile([128, 128], f32, tag="rc")
                    nc.vector.reciprocal(rc[:], pvs[:, 128:256])
                    nc.vector.tensor_mul(
                        ah[h][:, i * 128 : (i + 1) * 128], pvs[:, 0:128], rc[:]
                    )

                blocks = [(h, i) for h in range(HPC) for i in range(NST)]
                pending = []
                for (h, i) in blocks:
                    ps, nblk, j0 = emit_qk(h, i)
                    pt = emit_exp(ps, nblk)
                    pending.append((h, i, pt, nblk, j0))
                    if len(pending) > cfg["pipe"]:
                        emit_pv(*pending.pop(0))
                for it in pending:
                    emit_pv(*it)
                phps.close()

                if cfg["dump"]:
                    for h in range(HPC):
                        nc.sync.dma_start(dbg_qh[h * 128 : (h + 1) * 128, :], qh[h][:])
                        nc.sync.dma_start(dbg_kh[h * 128 : (h + 1) * 128, :], kh[h][:])
                        nc.sync.dma_start(dbg_ah[h * 128 : (h + 1) * 128, :], ah[h][:])
                    for j in range(NST):
                        nc.sync.dma_start(dbg_v[j * 128 : (j + 1) * 128, :], v16[j][:])

                # ------------- Phase C: output projection -------------
                if "C" in cfg["phases"]:
                    ob_pool = ph.enter_context(
                        tc.tile_pool(name="ob", bufs=cfg["ob_bufs"])
                    )
                    psc_pool = ph.enter_context(
                        tc.tile_pool(name="psc", bufs=cfg["psc_bufs"], space="PSUM")
                    )
                    for st_i in range(NST):
                        for mc in range(HIDDEN // 512):
                            ps = psc_pool.tile([128, 512], f32, tag="psc")
                            for h in range(HPC):
                                nc.tensor.matmul(
                                    ps[:],
                                    ah[h][:, st_i * 128 : (st_i + 1) * 128],
                                    wo_tiles[h][:, mc * 512 : (mc + 1) * 512],
                                    start=(h == 0),
                                    stop=(h == HPC - 1),
                                )
                            ob = ob_pool.tile([128, 512], f16, tag="ob")
                            nc.vector.tensor_copy(ob[:], ps[:])
                            nc.sync.dma_start(
                                out_d[
                                    st_i * 128 : (st_i + 1) * 128,
                                    mc * 512 : (mc + 1) * 512,
                                ],
                                ob[:],
                            )

    nc.compile()
    return nc


def _get_module(repeat=1, cfg=None):
    key = ("nc", repeat, tuple(sorted((cfg or {}).items())))
    if key not in _CACHE:
        _CACHE[key] = _build_module(repeat, cfg)
    return _CACHE[key]


def make_in_maps(hidden_states, cos, sin, w_qkv, w_o):
    hidden_states = np.asarray(hidden_states, dtype=np.float32)
    cos = np.asarray(cos, dtype=np.float32)
    sin = np.asarray(sin, dtype=np.float32)
    w_qkv = np.asarray(w_qkv, dtype=np.float32)
    w_o = np.asarray(w_o, dtype=np.float32)

    cosT = np.ascontiguousarray(cos.T)  # [DH, S]
    sinT = np.ascontiguousarray(sin.T)
    sinS = sinT.copy()
    sinS[: DH // 2] *= -1.0  # fold rotate_half sign
    cq = (cosT * SCALE).astype(np.float32)
    sq = (sinS * SCALE).astype(np.float32)
    ck = cosT.astype(np.float32)
    sk = sinS.astype(np.float32)

    # boundary-cell masks (in-cell coords: jj = key row, qq = query col)
    jj = np.arange(128)[:, None]
    qq = np.arange(128)[None, :]
    mdiag = np.where(qq >= jj, 0.0, NEG).astype(np.float16)
    mwend = np.where(qq < jj, 0.0, NEG).astype(np.float16)
    idn = np.eye(128, dtype=np.float16)
    ones = np.ones((128, 128), dtype=np.float16)

    xTs = [np.ascontiguousarray(hidden_states[b].T).astype(np.float16)
           for b in range(B)]

    in_maps = []
    for c in range(N_CORES):
        b, hg = divmod(c, N_CORES // B)
        r0 = hg * HPC * DH
        wq = w_qkv[r0 : r0 + HPC * DH]
        wk = w_qkv[N_HEADS * DH + r0 : N_HEADS * DH + r0 + HPC * DH]
        wv = w_qkv[2 * N_HEADS * DH + r0 : 2 * N_HEADS * DH + r0 + HPC * DH]
        wTc = np.ascontiguousarray(
            np.concatenate([wq, wk, wv], axis=0).T
        ).astype(np.float16)
        woTc = np.ascontiguousarray(w_o[:, r0 : r0 + HPC * DH].T).astype(np.float16)
        in_maps.append(
            {
                "xT": xTs[b],
                "wT": wTc,
                "woT": woTc,
                "cosq": cq,
                "sinq": sq,
                "cosk": ck,
                "sink": sk,
                "mdiag": mdiag,
                "mwend": mwend,
                "idn": idn,
                "ones": ones,
            }
        )
    return in_maps


def gather(results):
    out = np.zeros((B, S, HIDDEN), dtype=np.float32)
    for c in range(N_CORES):
        b = c // (N_CORES // B)
        out[b] += results[c]["out"].astype(np.float32)
    return out


def kernel(hidden_states, cos, sin, w_qkv, w_o):
    from concourse.bass_utils import run_bass_kernel_spmd

    nc = _get_module()
    in_maps = make_in_maps(hidden_states, cos, sin, w_qkv, w_o)
    res = run_bass_kernel_spmd(nc, in_maps, list(range(N_CORES)))
    return gather(res.results)
